# revision 7
# baseline (speedup 1.0000x reference)
"""UniversalLinear (BitNet b1.58 mode) Trainium2 kernel.

y = Q_int8(RMSNorm(x)) @ Q_ternary(W)^T

Math notes driving the implementation:
  - RMSNorm scale rms_t cancels inside the activation quant:
      x_norm/q = (rms*x) / (rms*m/127) = x * 127/m      (m = absmax per token)
    so the device only needs per-token absmax (m) and sum-of-squares (for the
    final output scale), never a normalized copy of x.
  - x_q in [-127,127] and w_q in {-1,0,1} are exact in bf16, and all partial
    dot products are integers < 2^18, so a bf16 matmul with fp32 PSUM
    accumulation is exact integer arithmetic.
  - Rounding: bf16(x*inv + 384) == 384 + round_half_even(x*inv) for
    |x*inv| <= 127 (bf16 has 8 mantissa bits, so ulp == 1 in [256,512)).
    The +384 offset is subtracted after the transpose in a cheap bf16 op.
  - Per-token output scale: os_t = (m_t/127) * w_scale / sqrt(mean(x^2)+eps),
    applied to the PSUM result.
  - Ternary weight quant is per-tensor and tiny: done on the host, shipped as
    a pre-transposed bf16 [128, 4, 512] tensor.

Sharding: pure data-parallel over batch; each of the 8 cores processes
B/8 = 2 batches = 8192 tokens. No collectives.
"""

import numpy as np
import ml_dtypes

import concourse.bass as bass
import concourse.bacc as bacc
import concourse.mybir as mybir
import concourse.tile as tile
from concourse.bass_utils import run_bass_kernel_spmd

N_CORES = 8
B, S, D = 16, 4096, 512
TOK_PER_CORE = (B // N_CORES) * S          # 8192
N_TILES = TOK_PER_CORE // 128              # 64
KC = D // 128                              # 4 contraction chunks
EPS = 1e-8
MAGIC = 1536.0                             # 1.5 * 2^10: fp16 cast rounds to integer

F32 = mybir.dt.float32
BF16 = mybir.dt.float16  # 16-bit matmul dtype (fp16: ints<=2048 exact)
Alu = mybir.AluOpType
Act = mybir.ActivationFunctionType


def build_bass(n_tiles: int = N_TILES) -> bass.Bass:
    nc = bacc.Bacc("TRN2", target_bir_lowering=False, debug=False,
                   num_devices=N_CORES)

    x_d = nc.dram_tensor("x", [n_tiles, 128, D], F32, kind="ExternalInput")
    wqt_d = nc.dram_tensor("wqt", [128, KC, D], BF16, kind="ExternalInput")
    wsb_d = nc.dram_tensor("wsb", [128, 1], F32, kind="ExternalInput")
    y_d = nc.dram_tensor("y", [n_tiles, 128, D], F32, kind="ExternalOutput")

    with tile.TileContext(nc) as tc:
        with (
            tc.tile_pool(name="const", bufs=1) as constp,
            tc.tile_pool(name="io", bufs=6) as iop,
            tc.tile_pool(name="work", bufs=4) as workp,
            tc.tile_pool(name="stats", bufs=6) as statp,
            tc.tile_pool(name="psum", bufs=6, space="PSUM") as psump,
        ):
            wqt = constp.tile([128, KC, D], BF16)
            nc.sync.dma_start(wqt[:], wqt_d[:])
            wsb = constp.tile([128, 1], F32)
            nc.sync.dma_start(wsb[:], wsb_d[:])
            eps_t = constp.tile([128, 1], F32)
            nc.gpsimd.memset(eps_t[:], EPS)

            for i in range(n_tiles):
                xt = iop.tile([128, D], F32, tag="xt")
                nc.sync.dma_start(xt[:], x_d[i])

                # mean(x^2) via ACT: accum(Square(x/sqrt(512)))
                sq = workp.tile([128, D], F32, tag="sq")
                msq = statp.tile([128, 1], F32, tag="msq")
                nc.scalar.activation(sq[:], xt[:], Act.Square,
                                     scale=float(1.0 / np.sqrt(512.0)),
                                     accum_out=msq[:])
                # std = sqrt(mean + eps)
                std = statp.tile([128, 1], F32, tag="std")
                nc.scalar.activation(std[:], msq[:], Act.Sqrt, bias=eps_t[:])

                # per-token absmax and derived scales
                m = statp.tile([128, 1], F32, tag="m")
                nc.vector.tensor_reduce(m[:], xt[:], axis=mybir.AxisListType.X,
                                        op=Alu.max, apply_absolute_value=True)
                xs = statp.tile([128, 1], F32, tag="xs")      # q = m/127
                nc.vector.tensor_scalar(xs[:], m[:], float(1.0 / 127.0), None,
                                        Alu.mult)
                inv = statp.tile([128, 1], F32, tag="inv")    # 1/q
                nc.vector.reciprocal(inv[:], xs[:])
                rstd = statp.tile([128, 1], F32, tag="rstd")  # 1/std
                nc.vector.reciprocal(rstd[:], std[:])
                t1 = statp.tile([128, 1], F32, tag="t1")      # q * w_scale
                nc.vector.tensor_scalar(t1[:], xs[:], wsb[:], None, Alu.mult)
                os_ = statp.tile([128, 1], F32, tag="os")     # q*ws/std
                nc.vector.tensor_tensor(os_[:], t1[:], rstd[:], Alu.mult)

                # quantize: bf16(x*inv + 384) = xq + 384 exactly (GPSIMD)
                xq = workp.tile([128, D], BF16, tag="xq")
                nc.gpsimd.tensor_scalar(xq[:], xt[:], inv[:], MAGIC,
                                        Alu.mult, Alu.add)

                # transpose to [din, tok] via DMA xbar (one 16-bit xbar DMA
                # yields the [128, KC, 128] chunked-transpose layout directly)
                xqT = workp.tile([128, KC, 128], BF16, tag="xqT")
                nc.sync.dma_start(xqT[:], xq[:], transpose=True)

                # remove the +384 offset (exact in bf16)
                xqTf = workp.tile([128, KC, 128], BF16, tag="xqTf")
                nc.vector.tensor_scalar(xqTf[:], xqT[:], MAGIC, None,
                                        Alu.subtract)

                # integer matmul in bf16, accumulate over 4 K-chunks
                ps = psump.tile([128, D], F32, tag="ps")
                for j in range(KC):
                    nc.tensor.matmul(ps[:], xqTf[:, j, :], wqt[:, j, :],
                                     start=(j == 0), stop=(j == KC - 1))

                # apply per-token output scale, store
                yt = iop.tile([128, D], F32, tag="yt")
                nc.vector.tensor_scalar(yt[:], ps[:], os_[:], None, Alu.mult)
                nc.sync.dma_start(y_d[i], yt[:])

    nc.compile()
    return nc


def host_prep(weight: np.ndarray, norm_weight: np.ndarray):
    """Quantize the weight on the host (exact ternary + per-tensor scale)."""
    w = weight.astype(np.float64)
    ws = max(float(np.mean(np.abs(w))), EPS)
    wq = np.round(np.clip(w / ws, -1.0, 1.0))          # {-1, 0, +1}
    # pre-transposed chunks: wqt[p, j, o] = wq[o, j*128 + p]
    wqt = np.ascontiguousarray(
        wq.T.reshape(KC, 128, D).transpose(1, 0, 2)
    ).astype(np.float16)
    wsb = np.full((128, 1), np.float32(ws), dtype=np.float32)
    return wqt, wsb


_NC_CACHE: dict[int, bass.Bass] = {}


def _get_nc(n_tiles: int = N_TILES) -> bass.Bass:
    if n_tiles not in _NC_CACHE:
        _NC_CACHE[n_tiles] = build_bass(n_tiles)
    return _NC_CACHE[n_tiles]


def _run(x: np.ndarray, weight: np.ndarray, norm_weight: np.ndarray,
         trace: bool = False):
    wqt, wsb = host_prep(weight, norm_weight)
    nc = _get_nc()
    shards = x.reshape(N_CORES, N_TILES, 128, D)
    in_maps = [
        {"x": np.ascontiguousarray(shards[c]), "wqt": wqt, "wsb": wsb}
        for c in range(N_CORES)
    ]
    res = run_bass_kernel_spmd(nc, in_maps, list(range(N_CORES)), trace=trace)
    y = np.stack([res.results[c]["y"] for c in range(N_CORES)])
    return y.reshape(B, S, D).astype(np.float32, copy=False), res


def _reference_host(x, weight, norm_weight):
    # numpy fallback, only used if norm_weight is not all-ones
    x = x.astype(np.float32)
    rms = 1.0 / np.sqrt(np.mean(x * x, axis=-1, keepdims=True) + EPS)
    xn = x * rms * norm_weight.astype(np.float32)
    sc = np.maximum(np.max(np.abs(xn), axis=-1, keepdims=True), EPS) / 127.0
    xdq = np.round(np.clip(xn / sc, -128.0, 127.0)) * sc
    w = weight.astype(np.float32)
    ws = np.maximum(np.mean(np.abs(w)), EPS)
    wdq = np.round(np.clip(w / ws, -1.0, 1.0)) * ws
    return (xdq.reshape(-1, D) @ wdq.T).reshape(x.shape[:-1] + (D,))


def kernel(x: np.ndarray, weight: np.ndarray,
           norm_weight: np.ndarray) -> np.ndarray:
    if not np.all(norm_weight == 1.0):
        return _reference_host(x, weight, norm_weight).astype(np.float32)
    y, _ = _run(np.asarray(x, dtype=np.float32),
                np.asarray(weight, dtype=np.float32),
                np.asarray(norm_weight, dtype=np.float32))
    return y
